# revision 9
# baseline (speedup 1.0000x reference)
"""Single-head attention (B=4, S=4096, E=1024, H=64) on 8 TRN2 NeuronCores.

Sharding: core c -> (batch b = c//2, sequence half = c%2). Each core gets the
full batch-b sequence with its own query half rotated to the front
(x_perm = [own 2048 rows, other 2048 rows]) and computes the attention output
for its 2048 queries. K/V for the whole 4096-row sequence are computed locally
(softmax is invariant to key order, so the rotation is harmless). No
cross-core communication.

Matmuls run in bf16 (fp32 lowers to two LOW_HIGH PE passes on TRN2 — half
throughput); accumulation is fp32 in PSUM, and the softmax denominator +
final normalization stay fp32.  All matmuls are zero-padded to full
128x128 stationary tiles: masked sub-tile matmuls (K=64 / M=65) run with
the PE clock-gated at 1.2 GHz (HAM never sees them as activity), while
full tiles keep the array at 2.4 GHz; the padding costs no extra stream
cycles.

On-chip layout (transposed until the final projection):
  xT[e, s]         PE transposes of bf16 x tiles
  kqT/vT[d, s]     = W_qkv.T @ xT   (d on partitions; K before Q so kT is at
                                     partition base 0)
  scoresT[kj, qi]  = kq_j.T @ q2    (q2 zero-padded on partitions 64:128)
  expT             = exp(scoresT * 1/8)   ACT, PSUM fp32 -> SBUF bf16
  ctxT_aug[65, qi] += v_aug_j.T @ expT_j  (v_aug carries a ones column, so row
                                           64 accumulates the softmax denom)
  out[qi, e]       = (ctxT.T @ W_out) * recip(denom) + b_out
"""

import sys

import numpy as np

for _p in ("/opt/trn_rl_repo",):
    if _p not in sys.path:
        sys.path.insert(0, _p)

from contextlib import ExitStack

import concourse.bass as bass  # noqa: F401  (import keeps bass registered)
import concourse.mybir as mybir
import concourse.tile as tile
from concourse import bacc, masks
from concourse.bass_utils import run_bass_kernel_spmd

F32 = mybir.dt.float32
BF16 = mybir.dt.bfloat16
AF = mybir.ActivationFunctionType
ALU = mybir.AluOpType

B, S, E, H = 4, 4096, 1024, 64
D3 = 3 * H            # 192
SH = S // 2           # queries per core
N_CORES = 8
QC = 1024             # query chunk per inner pass (PSUM-sized)
SCALE = 0.125         # 1/sqrt(H)
ST = S // 128         # 32 sequence tiles
ETILES = E // 128     # 8 embedding tiles
WSTR = 256            # w_sb per-e-tile stride: [K|Q|V|0] columns


def _emit(nc, tc, x_ext, wq_ext, bq_ext, wo_ext, bo_ext, out_ext):
    with ExitStack() as top:
        const = top.enter_context(tc.tile_pool(name="const", bufs=1))

        ident = const.tile([128, 128], BF16)
        masks.make_identity(nc, ident[:])
        ones11 = const.tile([1, 1], F32)
        nc.vector.memset(ones11[:], 1.0)

        # Weights: DMA fp32 staging -> cast to bf16.
        # W_qkv e-tiles as [K | Q | V | zeros] so both output m-tiles are full
        # 128-wide (kT lands on partitions 0:64, vT on 0:64 of m2).
        with tc.tile_pool(name="wstage", bufs=1) as wstage:
            w32 = wstage.tile([128, ETILES * WSTR], F32)
            nc.vector.memset(w32[:], 0.0)
            for e in range(ETILES):
                rows = slice(e * 128, (e + 1) * 128)
                o = e * WSTR
                nc.sync.dma_start(w32[:, o : o + 64], wq_ext[rows, 64:128])
                nc.sync.dma_start(w32[:, o + 64 : o + 128], wq_ext[rows, 0:64])
                nc.sync.dma_start(w32[:, o + 128 : o + 192], wq_ext[rows, 128:192])
            w_sb = const.tile([128, ETILES * WSTR], BF16)
            nc.vector.tensor_copy(w_sb[:], w32[:])
            wo32 = wstage.tile([H, E], F32)
            nc.sync.dma_start(wo32[:], wo_ext[:, :])
            wo_sb = const.tile([128, E], BF16)  # rows 64:128 zero (K-padding)
            nc.vector.memset(wo_sb[:], 0.0)
            nc.vector.tensor_copy(wo_sb[0:64, :], wo32[:])

        bkq = const.tile([128, 1], F32)  # [b_k ; b_q]
        nc.sync.dma_start(bkq[0:64, :], bq_ext[64:128].unsqueeze(1))
        nc.sync.dma_start(bkq[64:128, :], bq_ext[0:64].unsqueeze(1))
        bv = const.tile([64, 1], F32)
        nc.sync.dma_start(bv[:], bq_ext[128:192].unsqueeze(1))
        bo_bc = const.tile([128, E], F32)
        nc.sync.dma_start(bo_bc[:], bo_ext.unsqueeze(0).partition_broadcast(128))

        # Persistent activations (bf16 matmul operands)
        kq_sb = const.tile([128, S], BF16)   # [kT ; qT] by partition halves
        vT_sb = const.tile([64, S], BF16)
        q2_sb = const.tile([128, SH], BF16)  # qT on 0:64, zeros on 64:128
        nc.vector.memset(q2_sb[:], 0.0)
        # per kj-tile: [v(64) | ones(1) | zeros(63)] -> full 128-wide lhsT
        v_aug = const.tile([128, ST * 128], BF16)
        nc.vector.memset(v_aug[:], 0.0)
        for kj in range(ST):
            nc.vector.memset(v_aug[:, kj * 128 + 64 : kj * 128 + 65], 1.0)

        # ---- Phase A: cast x to bf16, transpose via DMA xbar (DRAM
        # roundtrip), project to kT/qT/vT, per s-chunk ---------------------
        with ExitStack() as pa:
            dram = pa.enter_context(tc.tile_pool(name="dram", bufs=1, space="DRAM"))
            xb_dram = dram.tile([S, E], BF16)
            vt_dram = dram.tile([64, S], BF16)
            xsb = pa.enter_context(tc.tile_pool(name="xsb", bufs=4))
            xbp = pa.enter_context(tc.tile_pool(name="xbp", bufs=8))
            xTp = pa.enter_context(tc.tile_pool(name="xTp", bufs=1))
            m1p = pa.enter_context(tc.tile_pool(name="m1p", bufs=2, space="PSUM"))
            m2p = pa.enter_context(tc.tile_pool(name="m2p", bufs=2, space="PSUM"))
            for sc in range(4):                # s chunks of 1024 (8 s-tiles)
                for si in range(8):
                    st = sc * 8 + si
                    t32 = xsb.tile([128, E], F32)
                    nc.sync.dma_start(t32[:], x_ext[st * 128 : (st + 1) * 128, :])
                    tb = xbp.tile([128, E], BF16)
                    if si % 2 == 0:
                        nc.vector.tensor_copy(tb[:], t32[:])
                    else:
                        nc.scalar.copy(tb[:], t32[:])
                    nc.sync.dma_start(xb_dram[st * 128 : (st + 1) * 128, :], tb[:])
                xT_sc = xTp.tile([128, ETILES * 1024], BF16)  # xT[:, sc rows]
                rows = slice(sc * 1024, (sc + 1) * 1024)
                for e in range(ETILES):
                    nc.sync.dma_start_transpose(
                        xT_sc[:, e * 1024 : (e + 1) * 1024],
                        xb_dram[rows, e * 128 : (e + 1) * 128],
                    )

                m1 = m1p.tile([128, 1024], F32)
                m2 = m2p.tile([128, 1024], F32)
                for e in range(ETILES):
                    lhs1 = w_sb[:, e * WSTR : e * WSTR + 128]
                    lhs2 = w_sb[:, e * WSTR + 128 : e * WSTR + 256]
                    for n in range(2):
                        rhs = xT_sc[:, e * 1024 + n * 512 : e * 1024 + (n + 1) * 512]
                        nc.tensor.matmul(
                            m1[:, n * 512 : (n + 1) * 512], lhs1, rhs,
                            start=(e == 0), stop=(e == ETILES - 1),
                        )
                        nc.tensor.matmul(
                            m2[:, n * 512 : (n + 1) * 512], lhs2, rhs,
                            start=(e == 0), stop=(e == ETILES - 1),
                        )
                cols = slice(sc * 1024, (sc + 1) * 1024)
                nc.vector.tensor_scalar_add(kq_sb[:, cols], m1[:], bkq[:])
                nc.scalar.add(vT_sb[:, cols], m2[0:64, :], bv[:])
                if sc < 2:  # queries live in s 0:2048
                    nc.sync.dma_start(q2_sb[0:64, cols], kq_sb[64:128, cols])
                # v natural tiles for this chunk (kj = sc*8 .. sc*8+7),
                # via DMA xbar transpose through DRAM
                nc.sync.dma_start(vt_dram[:, cols], vT_sb[:, cols])
                for kj in range(sc * 8, (sc + 1) * 8):
                    nc.sync.dma_start_transpose(
                        v_aug[:, kj * 128 : kj * 128 + 64],
                        vt_dram[:, kj * 128 : (kj + 1) * 128],
                    )

        # ---- Phase B/C: attention + output projection -------------------
        with ExitStack() as pb:
            sps = pb.enter_context(tc.tile_pool(name="sps", bufs=2, space="PSUM"))
            cps = pb.enter_context(tc.tile_pool(name="cps", bufs=1, space="PSUM"))
            ops = pb.enter_context(tc.tile_pool(name="ops", bufs=2, space="PSUM"))
            expp = pb.enter_context(tc.tile_pool(name="expp", bufs=4))
            ctxp = pb.enter_context(tc.tile_pool(name="ctxp", bufs=2))
            rsp = pb.enter_context(tc.tile_pool(name="rsp", bufs=2))
            outp = pb.enter_context(tc.tile_pool(name="outp", bufs=3))

            for qc in range(SH // QC):
                q0 = qc * QC
                ctx = cps.tile([128, QC], F32)  # rows 0:64 ctx, 64 denom
                for kj in range(ST):
                    sc_ps = sps.tile([128, QC], F32)
                    lhs_k = kq_sb[:, kj * 128 : (kj + 1) * 128]
                    for n in range(QC // 512):
                        nc.tensor.matmul(
                            sc_ps[:, n * 512 : (n + 1) * 512],
                            lhs_k,
                            q2_sb[:, q0 + n * 512 : q0 + (n + 1) * 512],
                        )
                    ex = expp.tile([128, QC], BF16)
                    nc.scalar.activation(ex[:], sc_ps[:], AF.Exp, scale=SCALE)
                    lhs_v = v_aug[:, kj * 128 : (kj + 1) * 128]
                    for n in range(QC // 512):
                        nc.tensor.matmul(
                            ctx[:, n * 512 : (n + 1) * 512],
                            lhs_v,
                            ex[:, n * 512 : (n + 1) * 512],
                            start=(kj == 0), stop=(kj == ST - 1),
                            skip_group_check=True,
                        )

                ctx_sb = ctxp.tile([65, QC], F32, tag="ctx32")
                nc.vector.tensor_copy(ctx_sb[:], ctx[0:65, :])
                ctx_b16 = ctxp.tile([128, QC], BF16, tag="ctx16")
                nc.vector.memset(ctx_b16[64:128, :], 0.0)
                nc.vector.tensor_copy(ctx_b16[0:64, :], ctx_sb[0:64, :])
                rs_row = rsp.tile([1, QC], F32, tag="rsrow")
                nc.sync.dma_start(rs_row[:], ctx_sb[64:65, :])

                rs_ps = ops.tile([128, QC // 128], F32, tag="op")
                for c in range(QC // 128):
                    nc.tensor.matmul(
                        rs_ps[:, c : c + 1],
                        rs_row[0:1, c * 128 : (c + 1) * 128],
                        ones11[:],
                    )
                recip = rsp.tile([128, QC // 128], F32, tag="recip")
                nc.vector.reciprocal(recip[:], rs_ps[:])

                for c in range(QC // 128):
                    out_sb = outp.tile([128, E], F32)
                    for n in range(2):
                        op = ops.tile([128, 512], F32, tag="op")
                        nc.tensor.matmul(
                            op[:],
                            ctx_b16[:, c * 128 : (c + 1) * 128],
                            wo_sb[:, n * 512 : (n + 1) * 512],
                        )
                        nc.vector.scalar_tensor_tensor(
                            out_sb[:, n * 512 : (n + 1) * 512],
                            op[:],
                            recip[:, c : c + 1],
                            bo_bc[:, n * 512 : (n + 1) * 512],
                            op0=ALU.mult,
                            op1=ALU.add,
                        )
                    nc.sync.dma_start(
                        out_ext[q0 + c * 128 : q0 + (c + 1) * 128, :], out_sb[:]
                    )


_NC = None


def _get_nc():
    global _NC
    if _NC is None:
        nc = bacc.Bacc("TRN2", target_bir_lowering=False, debug=False,
                       num_devices=N_CORES)
        x_ext = nc.dram_tensor("x", [S, E], F32, kind="ExternalInput").ap()
        wq_ext = nc.dram_tensor("w_qkv", [E, D3], F32, kind="ExternalInput").ap()
        bq_ext = nc.dram_tensor("b_qkv", [D3], F32, kind="ExternalInput").ap()
        wo_ext = nc.dram_tensor("w_out", [H, E], F32, kind="ExternalInput").ap()
        bo_ext = nc.dram_tensor("b_out", [E], F32, kind="ExternalInput").ap()
        out_ext = nc.dram_tensor("out", [SH, E], F32, kind="ExternalOutput").ap()
        with tile.TileContext(nc) as tc:
            _emit(nc, tc, x_ext, wq_ext, bq_ext, wo_ext, bo_ext, out_ext)
        nc.compile()
        _NC = nc
    return _NC


last_results = None
last_tmpdir = None


def kernel(x, W_qkv, b_qkv, W_out, b_out):
    nc = _get_nc()
    x = np.ascontiguousarray(x, dtype=np.float32)
    shared = {
        "w_qkv": np.ascontiguousarray(W_qkv, dtype=np.float32),
        "b_qkv": np.ascontiguousarray(b_qkv, dtype=np.float32),
        "w_out": np.ascontiguousarray(W_out, dtype=np.float32),
        "b_out": np.ascontiguousarray(b_out, dtype=np.float32),
    }
    in_maps = []
    for c in range(N_CORES):
        b, h = divmod(c, 2)
        xb = x[b]
        xp = xb if h == 0 else np.ascontiguousarray(np.roll(xb, -SH, axis=0))
        in_maps.append({"x": xp, **shared})

    import os
    import tempfile

    tmpdir = os.environ.get("ATTN_TRACE_DIR") or tempfile.mkdtemp(prefix="attn_trace_")
    res = run_bass_kernel_spmd(nc, in_maps, core_ids=list(range(N_CORES)), tmpdir=tmpdir)
    global last_results, last_tmpdir
    last_results = res
    last_tmpdir = tmpdir

    out = np.empty((B, S, E), dtype=np.float32)
    for c in range(N_CORES):
        b, h = divmod(c, 2)
        out[b, h * SH : (h + 1) * SH] = res.results[c]["out"]
    return out


# revision 12
# speedup vs baseline: 1.7499x; 1.7499x over previous
"""Single-head attention (B=4, S=4096, E=1024, H=64) on 8 TRN2 NeuronCores.

Sharding: core c -> (batch b = c//2, sequence half h = c%2). Each core receives
only its own 2048-row x half, computes Q/K/V for it, and the core pair
(2b, 2b+1) exchanges K/V halves with a 2-rank AllGather (two chunked AGs,
overlapped with the projection and the first attention tiles). Every core then
holds K/V for the full 4096-row sequence in global order and computes
attention for its 2048 queries.

Matmuls run in bf16 (fp32 lowers to two LOW_HIGH PE passes on TRN2 — half
throughput); accumulation is fp32 in PSUM, the softmax denominator and the
normalization stay fp32. All matmuls are zero-padded to full 128x128
stationary tiles: masked sub-tile matmuls (K=64 / M=65) leave the PE
clock-gated at 1.2 GHz (HAM does not see them as activity), while full tiles
keep it at 2.4 GHz; the padding costs no extra stream cycles.

Output projection: W_out is padded with b_out as row 64 and the bf16 context
carries the softmax denominator in row 64, so (ctx_aug.T @ W_out_aug) *
recip(denom) applies scale and bias in one pass (denom * recip == 1).
"""

import sys

import numpy as np

for _p in ("/opt/trn_rl_repo",):
    if _p not in sys.path:
        sys.path.insert(0, _p)

from contextlib import ExitStack

import concourse.bass as bass  # noqa: F401  (import keeps bass registered)
import concourse.mybir as mybir
import concourse.tile as tile
from concourse import bacc, masks
from concourse.bass_utils import run_bass_kernel_spmd

F32 = mybir.dt.float32
BF16 = mybir.dt.bfloat16
AF = mybir.ActivationFunctionType
ALU = mybir.AluOpType

B, S, E, H = 4, 4096, 1024, 64
D3 = 3 * H            # 192
SH = S // 2           # queries per core
N_CORES = 8
QC = 1024             # query chunk per inner pass (PSUM-sized)
SCALE = 0.125         # 1/sqrt(H)
ST = S // 128         # 32 kj tiles over the full sequence
ETILES = E // 128     # 8 embedding tiles
WSTR = 256            # w_sb per-e-tile stride: [K|Q|V|0] columns
WKV = 64 * 1024 + 64 * 1024  # AG payload elems per chunk: kT[64,1024] + vT[64,1024]
REPLICA_GROUPS = [[0, 1], [2, 3], [4, 5], [6, 7]]


def _emit(nc, tc, x_ext, wq_ext, bq_ext, wo_ext, bo_ext, out_ext):
    with ExitStack() as top:
        const = top.enter_context(tc.tile_pool(name="const", bufs=1))

        ident = const.tile([128, 128], BF16)
        masks.make_identity(nc, ident[:])
        ones11 = const.tile([1, 1], F32)
        nc.gpsimd.memset(ones11[:], 1.0)

        # Weights: DMA fp32 staging -> cast to bf16.
        # W_qkv e-tiles as [K | Q | V | zeros]; both m-tiles are full 128 wide
        # (kT lands on partitions 0:64 of m1, vT on 0:64 of m2).
        with tc.tile_pool(name="wstage", bufs=1) as wstage:
            w32 = wstage.tile([128, ETILES * WSTR], F32)
            nc.gpsimd.memset(w32[:], 0.0)
            for e in range(ETILES):
                rows = slice(e * 128, (e + 1) * 128)
                o = e * WSTR
                nc.gpsimd.dma_start(w32[:, o : o + 64], wq_ext[rows, 64:128])
                nc.gpsimd.dma_start(w32[:, o + 64 : o + 128], wq_ext[rows, 0:64])
                nc.gpsimd.dma_start(w32[:, o + 128 : o + 192], wq_ext[rows, 128:192])
            w_sb = const.tile([128, ETILES * WSTR], BF16)
            nc.vector.tensor_copy(w_sb[:], w32[:])

            # W_out padded: rows 0:64 = W_out, row 64 = b_out, rows 65:128 = 0
            wo32 = wstage.tile([H, E], F32)
            nc.gpsimd.dma_start(wo32[:], wo_ext[:, :])
            bo16 = wstage.tile([1, E], BF16)
            bo32 = wstage.tile([1, E], F32)
            nc.gpsimd.dma_start(bo32[:], bo_ext.unsqueeze(0))
            nc.vector.tensor_copy(bo16[:], bo32[:])
            wo_sb = const.tile([128, E], BF16)
            nc.gpsimd.memset(wo_sb[:], 0.0)
            nc.vector.tensor_copy(wo_sb[0:64, :], wo32[:])
            nc.gpsimd.dma_start(wo_sb[64:65, :], bo16[:])

        bkq = const.tile([128, 1], F32)  # [b_k ; b_q]
        nc.gpsimd.dma_start(bkq[0:64, :], bq_ext[64:128].unsqueeze(1))
        nc.gpsimd.dma_start(bkq[64:128, :], bq_ext[0:64].unsqueeze(1))
        bv = const.tile([64, 1], F32)
        nc.gpsimd.dma_start(bv[:], bq_ext[128:192].unsqueeze(1))

        # Persistent bf16 matmul operands (global kv order on the free axis)
        kt_sb = const.tile([128, S], BF16)   # kT on 0:64, zeros on 64:128
        nc.gpsimd.memset(kt_sb[64:128, :], 0.0)
        vT_sb = const.tile([64, S], BF16)
        q2_sb = const.tile([128, SH], BF16)  # qT on 0:64, zeros on 64:128
        nc.gpsimd.memset(q2_sb[:], 0.0)
        # per kj-tile lhsT: [v(64) | ones(1) | junk(63)] -> full 128-wide
        v_aug = const.tile([128, ST * 128], BF16)
        nc.gpsimd.memset(v_aug[:], 0.0)
        for kj in range(ST):
            nc.gpsimd.memset(v_aug[:, kj * 128 + 64 : kj * 128 + 65], 1.0)

        # Collective bounce buffers (per AG chunk)
        dram = top.enter_context(tc.tile_pool(name="ccdram", bufs=1, space="DRAM"))
        cc_in = [dram.tile([1, WKV], BF16, name=f"cc_in{c}") for c in range(2)]
        cc_out = [dram.tile([2, WKV], BF16, name=f"cc_out{c}") for c in range(2)]

        # ---- Phase A: per own-half s-chunk: cast, PE-transpose, project,
        # stage K/V into the pair AllGather --------------------------------
        with ExitStack() as pa:
            xsb = pa.enter_context(tc.tile_pool(name="xsb", bufs=4))
            xbp = pa.enter_context(tc.tile_pool(name="xbp", bufs=8))
            xTp = pa.enter_context(tc.tile_pool(name="xTp", bufs=2))
            stg = pa.enter_context(tc.tile_pool(name="stg", bufs=2))
            xtp = pa.enter_context(tc.tile_pool(name="xtp", bufs=2, space="PSUM"))
            m1p = pa.enter_context(tc.tile_pool(name="m1p", bufs=1, space="PSUM"))
            m2p = pa.enter_context(tc.tile_pool(name="m2p", bufs=1, space="PSUM"))
            vps = pa.enter_context(tc.tile_pool(name="vps", bufs=2, space="PSUM"))

            for sc in range(2):                # own-half s chunks of 1024
                xbs = []
                for si in range(8):
                    st = sc * 8 + si
                    t32 = xsb.tile([128, E], F32)
                    nc.sync.dma_start(t32[:], x_ext[st * 128 : (st + 1) * 128, :])
                    tb = xbp.tile([128, E], BF16)
                    if si % 2 == 0:
                        nc.vector.tensor_copy(tb[:], t32[:])
                    else:
                        nc.scalar.copy(tb[:], t32[:])
                    xbs.append(tb)
                xT_sc = xTp.tile([128, ETILES * 1024], BF16)
                for e in range(ETILES):
                    p = xtp.tile([128, 1024], BF16)
                    for si in range(8):
                        nc.tensor.transpose(
                            p[:, si * 128 : (si + 1) * 128],
                            xbs[si][:, e * 128 : (e + 1) * 128],
                            ident[:],
                        )
                    dst = xT_sc[:, e * 1024 : (e + 1) * 1024]
                    if e % 2 == 0:
                        nc.vector.tensor_copy(dst, p[:])
                    else:
                        nc.scalar.copy(dst, p[:])

                m1 = m1p.tile([128, 1024], F32)
                m2 = m2p.tile([128, 1024], F32)
                for e in range(ETILES):
                    lhs1 = w_sb[:, e * WSTR : e * WSTR + 128]
                    lhs2 = w_sb[:, e * WSTR + 128 : e * WSTR + 256]
                    for n in range(2):
                        rhs = xT_sc[:, e * 1024 + n * 512 : e * 1024 + (n + 1) * 512]
                        nc.tensor.matmul(
                            m1[:, n * 512 : (n + 1) * 512], lhs1, rhs,
                            start=(e == 0), stop=(e == ETILES - 1),
                        )
                        nc.tensor.matmul(
                            m2[:, n * 512 : (n + 1) * 512], lhs2, rhs,
                            start=(e == 0), stop=(e == ETILES - 1),
                        )
                kq = stg.tile([128, 1024], BF16, tag="kq")
                nc.vector.tensor_scalar_add(kq[:], m1[:], bkq[:])
                vst = stg.tile([64, 1024], BF16, tag="vst")
                nc.scalar.add(vst[:], m2[0:64, :], bv[:])

                # stage into the AG and keep qT locally
                nc.gpsimd.dma_start(cc_in[sc][0, 0 : 64 * 1024], kq[0:64, :])
                nc.gpsimd.dma_start(cc_in[sc][0, 64 * 1024 : WKV], vst[:])
                nc.sync.dma_start(
                    q2_sb[0:64, sc * 1024 : (sc + 1) * 1024], kq[64:128, :]
                )
                nc.gpsimd.collective_compute(
                    "AllGather",
                    ALU.bypass,
                    replica_groups=REPLICA_GROUPS,
                    ins=[cc_in[sc].opt()],
                    outs=[cc_out[sc].opt()],
                )
                # unpack both halves in global order; r = global half index
                for r in range(2):
                    cols = slice(r * SH + sc * 1024, r * SH + (sc + 1) * 1024)
                    nc.sync.dma_start(
                        kt_sb[0:64, cols],
                        cc_out[sc][r, 0 : 64 * 1024].rearrange("(p f) -> p f", p=64),
                    )
                    nc.sync.dma_start(
                        vT_sb[:, cols],
                        cc_out[sc][r, 64 * 1024 : WKV].rearrange("(p f) -> p f", p=64),
                    )
                # v natural tiles covered by this AG chunk
                for r in range(2):
                    for t in range(8):
                        kj = r * 16 + sc * 8 + t
                        p = vps.tile([128, 64], BF16)
                        nc.tensor.transpose(
                            p[:],
                            vT_sb[:, kj * 128 : (kj + 1) * 128],
                            ident[0:64, 0:64],
                        )
                        nc.vector.tensor_copy(
                            v_aug[:, kj * 128 : kj * 128 + 64], p[:]
                        )

        # kj visit order: tiles covered by AG#0 first, then AG#1
        kj_order = (
            list(range(0, 8)) + list(range(16, 24))
            + list(range(8, 16)) + list(range(24, 32))
        )

        # ---- Phase B/C: attention + output projection -------------------
        with ExitStack() as pb:
            sps = pb.enter_context(tc.tile_pool(name="sps", bufs=2, space="PSUM"))
            cps = pb.enter_context(tc.tile_pool(name="cps", bufs=1, space="PSUM"))
            ops = pb.enter_context(tc.tile_pool(name="ops", bufs=2, space="PSUM"))
            expp = pb.enter_context(tc.tile_pool(name="expp", bufs=4))
            ctxp = pb.enter_context(tc.tile_pool(name="ctxp", bufs=2))
            rsp = pb.enter_context(tc.tile_pool(name="rsp", bufs=2))
            outp = pb.enter_context(tc.tile_pool(name="outp", bufs=3))

            for qc in range(SH // QC):
                q0 = qc * QC
                ctx = cps.tile([128, QC], F32)  # rows 0:64 ctx, row 64 denom
                for i, kj in enumerate(kj_order):
                    sc_ps = sps.tile([128, QC], F32)
                    lhs_k = kt_sb[:, kj * 128 : (kj + 1) * 128]
                    for n in range(QC // 512):
                        nc.tensor.matmul(
                            sc_ps[:, n * 512 : (n + 1) * 512],
                            lhs_k,
                            q2_sb[:, q0 + n * 512 : q0 + (n + 1) * 512],
                        )
                    ex = expp.tile([128, QC], BF16)
                    nc.scalar.activation(ex[:], sc_ps[:], AF.Exp, scale=SCALE)
                    lhs_v = v_aug[:, kj * 128 : (kj + 1) * 128]
                    for n in range(QC // 512):
                        nc.tensor.matmul(
                            ctx[:, n * 512 : (n + 1) * 512],
                            lhs_v,
                            ex[:, n * 512 : (n + 1) * 512],
                            start=(i == 0), stop=(i == ST - 1),
                            skip_group_check=True,
                        )

                ctx_sb = ctxp.tile([65, QC], F32, tag="ctx32")
                nc.vector.tensor_copy(ctx_sb[:], ctx[0:65, :])
                # bf16 context with the denominator kept as row 64 (so the
                # b_out row of wo_sb gets scaled by denom*recip == 1)
                ctx_b16 = ctxp.tile([128, QC], BF16, tag="ctx16")
                nc.gpsimd.memset(ctx_b16[64:128, :], 0.0)
                nc.vector.tensor_copy(ctx_b16[0:65, :], ctx_sb[:])
                rs_row = rsp.tile([1, QC], F32, tag="rsrow")
                nc.sync.dma_start(rs_row[:], ctx_sb[64:65, :])

                rs_ps = ops.tile([128, QC // 128], F32, tag="op")
                for c in range(QC // 128):
                    nc.tensor.matmul(
                        rs_ps[:, c : c + 1],
                        rs_row[0:1, c * 128 : (c + 1) * 128],
                        ones11[:],
                    )
                recip = rsp.tile([128, QC // 128], F32, tag="recip")
                nc.vector.reciprocal(recip[:], rs_ps[:])

                for c in range(QC // 128):
                    out_sb = outp.tile([128, E], F32)
                    for n in range(2):
                        op = ops.tile([128, 512], F32, tag="op")
                        nc.tensor.matmul(
                            op[:],
                            ctx_b16[:, c * 128 : (c + 1) * 128],
                            wo_sb[:, n * 512 : (n + 1) * 512],
                        )
                        nc.vector.tensor_scalar_mul(
                            out_sb[:, n * 512 : (n + 1) * 512],
                            op[:],
                            recip[:, c : c + 1],
                        )
                    nc.sync.dma_start(
                        out_ext[q0 + c * 128 : q0 + (c + 1) * 128, :], out_sb[:]
                    )


_NC = None


def _get_nc():
    global _NC
    if _NC is None:
        nc = bacc.Bacc("TRN2", target_bir_lowering=False, debug=False,
                       num_devices=N_CORES)
        x_ext = nc.dram_tensor("x", [SH, E], F32, kind="ExternalInput").ap()
        wq_ext = nc.dram_tensor("w_qkv", [E, D3], F32, kind="ExternalInput").ap()
        bq_ext = nc.dram_tensor("b_qkv", [D3], F32, kind="ExternalInput").ap()
        wo_ext = nc.dram_tensor("w_out", [H, E], F32, kind="ExternalInput").ap()
        bo_ext = nc.dram_tensor("b_out", [E], F32, kind="ExternalInput").ap()
        out_ext = nc.dram_tensor("out", [SH, E], F32, kind="ExternalOutput").ap()
        with tile.TileContext(nc) as tc:
            _emit(nc, tc, x_ext, wq_ext, bq_ext, wo_ext, bo_ext, out_ext)
        nc.compile()
        _NC = nc
    return _NC


last_results = None
last_tmpdir = None


def kernel(x, W_qkv, b_qkv, W_out, b_out):
    nc = _get_nc()
    x = np.ascontiguousarray(x, dtype=np.float32)
    shared = {
        "w_qkv": np.ascontiguousarray(W_qkv, dtype=np.float32),
        "b_qkv": np.ascontiguousarray(b_qkv, dtype=np.float32),
        "w_out": np.ascontiguousarray(W_out, dtype=np.float32),
        "b_out": np.ascontiguousarray(b_out, dtype=np.float32),
    }
    in_maps = []
    for c in range(N_CORES):
        b, h = divmod(c, 2)
        xp = np.ascontiguousarray(x[b, h * SH : (h + 1) * SH])
        in_maps.append({"x": xp, **shared})

    import os
    import tempfile

    tmpdir = os.environ.get("ATTN_TRACE_DIR") or tempfile.mkdtemp(prefix="attn_trace_")
    res = run_bass_kernel_spmd(nc, in_maps, core_ids=list(range(N_CORES)), tmpdir=tmpdir)
    global last_results, last_tmpdir
    last_results = res
    last_tmpdir = tmpdir

    out = np.empty((B, S, E), dtype=np.float32)
    for c in range(N_CORES):
        b, h = divmod(c, 2)
        out[b, h * SH : (h + 1) * SH] = res.results[c]["out"]
    return out


# revision 15
# speedup vs baseline: 1.8046x; 1.0312x over previous
"""Single-head attention (B=4, S=4096, E=1024, H=64) on 8 TRN2 NeuronCores.

Sharding: core c -> (batch b = c//2, sequence half h = c%2). Each core receives
only its own 2048-row x half, computes Q/K/V for it, and the core pair
(2b, 2b+1) exchanges K/V halves with a 2-rank AllGather (two chunked AGs,
overlapped with the projection and the first attention tiles). Every core then
holds K/V for the full 4096-row sequence in global order and computes
attention for its 2048 queries.

Matmuls run in bf16 (fp32 lowers to two LOW_HIGH PE passes on TRN2 — half
throughput); accumulation is fp32 in PSUM, the softmax denominator and the
normalization stay fp32. All matmuls are zero-padded to full 128x128
stationary tiles: masked sub-tile matmuls (K=64 / M=65) leave the PE
clock-gated at 1.2 GHz (HAM does not see them as activity), while full tiles
keep it at 2.4 GHz; the padding costs no extra stream cycles.

Output projection: W_out is padded with b_out as row 64 and the bf16 context
carries the softmax denominator in row 64, so (ctx_aug.T @ W_out_aug) *
recip(denom) applies scale and bias in one pass (denom * recip == 1).
"""

import sys

import numpy as np

for _p in ("/opt/trn_rl_repo",):
    if _p not in sys.path:
        sys.path.insert(0, _p)

from contextlib import ExitStack

import concourse.bass as bass  # noqa: F401  (import keeps bass registered)
import concourse.mybir as mybir
import concourse.tile as tile
from concourse import bacc, masks
from concourse.bass_utils import run_bass_kernel_spmd

F32 = mybir.dt.float32
BF16 = mybir.dt.bfloat16
AF = mybir.ActivationFunctionType
ALU = mybir.AluOpType

B, S, E, H = 4, 4096, 1024, 64
D3 = 3 * H            # 192
SH = S // 2           # queries per core
N_CORES = 8
QC = 1024             # query chunk per inner pass (PSUM-sized)
SCALE = 0.125         # 1/sqrt(H)
ST = S // 128         # 32 kj tiles over the full sequence
ETILES = E // 128     # 8 embedding tiles
WSTR = 256            # w_sb per-e-tile stride: [K|Q|V|0] columns
WKV = 64 * 1024 + 64 * 1024  # AG payload elems per chunk: kT[64,1024] + vT[64,1024]
REPLICA_GROUPS = [[0, 1], [2, 3], [4, 5], [6, 7]]


def _emit(nc, tc, x_ext, wq_ext, bq_ext, wo_ext, bo_ext, out_ext):
    with ExitStack() as top:
        const = top.enter_context(tc.tile_pool(name="const", bufs=1))

        # Critical path first: identity (needed by the first transposes) and
        # the QKV weight staging. Everything phase-B-only is emitted after
        # phase A so it never blocks the gpsimd/DVE queues early.
        ident = const.tile([128, 128], BF16)
        masks.make_identity(nc, ident[:])

        # Weights: DMA fp32 staging -> cast to bf16.
        # W_qkv e-tiles as [K | Q | V | junk]; both m-tiles are full 128 wide
        # (kT lands on partitions 0:64 of m1, vT on 0:64 of m2; the junk
        # columns only feed the unused rows 64:128 of m2).
        wstage_ctx = ExitStack()
        wstage = wstage_ctx.enter_context(tc.tile_pool(name="wstage", bufs=1))
        w32 = wstage.tile([128, ETILES * WSTR], F32)
        for e in range(ETILES):
            rows = slice(e * 128, (e + 1) * 128)
            o = e * WSTR
            nc.sync.dma_start(w32[:, o : o + 64], wq_ext[rows, 64:128])
            nc.sync.dma_start(w32[:, o + 64 : o + 128], wq_ext[rows, 0:64])
            nc.sync.dma_start(w32[:, o + 128 : o + 192], wq_ext[rows, 128:192])
        w_sb = const.tile([128, ETILES * WSTR], BF16)
        nc.vector.tensor_copy(w_sb[:], w32[:])

        bkq = const.tile([128, 1], F32)  # [b_k ; b_q]
        nc.gpsimd.dma_start(bkq[0:64, :], bq_ext[64:128].unsqueeze(1))
        nc.gpsimd.dma_start(bkq[64:128, :], bq_ext[0:64].unsqueeze(1))
        bv = const.tile([64, 1], F32)
        nc.gpsimd.dma_start(bv[:], bq_ext[128:192].unsqueeze(1))

        # Persistent bf16 matmul operands (global kv order on the free axis)
        kt_sb = const.tile([128, S], BF16)   # kT on 0:64, zeros on 64:128
        vT_sb = const.tile([64, S], BF16)
        q2_sb = const.tile([128, SH], BF16)  # qT on 0:64, zeros on 64:128
        nc.gpsimd.memset(q2_sb[:], 0.0)
        v_aug = const.tile([128, ST * 128], BF16)
        ones11 = const.tile([1, 1], F32)
        wo_sb = const.tile([128, E], BF16)

        # Collective bounce buffers (per AG chunk)
        dram = top.enter_context(tc.tile_pool(name="ccdram", bufs=1, space="DRAM"))
        cc_in = [dram.tile([1, WKV], BF16, name=f"cc_in{c}") for c in range(2)]
        cc_out = [dram.tile([2, WKV], BF16, name=f"cc_out{c}") for c in range(2)]

        # ---- Phase A: per own-half s-chunk: cast, PE-transpose, project,
        # stage K/V into the pair AllGather --------------------------------
        with ExitStack() as pa:
            xsb = pa.enter_context(tc.tile_pool(name="xsb", bufs=4))
            xbp = pa.enter_context(tc.tile_pool(name="xbp", bufs=8))
            xTp = pa.enter_context(tc.tile_pool(name="xTp", bufs=2))
            stg = pa.enter_context(tc.tile_pool(name="stg", bufs=2))
            xtp = pa.enter_context(tc.tile_pool(name="xtp", bufs=2, space="PSUM"))
            m1p = pa.enter_context(tc.tile_pool(name="m1p", bufs=1, space="PSUM"))
            m2p = pa.enter_context(tc.tile_pool(name="m2p", bufs=1, space="PSUM"))
            vps = pa.enter_context(tc.tile_pool(name="vps", bufs=2, space="PSUM"))

            for sc in range(2):                # own-half s chunks of 1024
                xbs = []
                for si in range(8):
                    st = sc * 8 + si
                    t32 = xsb.tile([128, E], F32)
                    nc.sync.dma_start(t32[:], x_ext[st * 128 : (st + 1) * 128, :])
                    tb = xbp.tile([128, E], BF16)
                    if si % 2 == 0:
                        nc.vector.tensor_copy(tb[:], t32[:])
                    else:
                        nc.scalar.copy(tb[:], t32[:])
                    xbs.append(tb)
                xT_sc = xTp.tile([128, ETILES * 1024], BF16)
                for e in range(ETILES):
                    p = xtp.tile([128, 1024], BF16)
                    for si in range(8):
                        nc.tensor.transpose(
                            p[:, si * 128 : (si + 1) * 128],
                            xbs[si][:, e * 128 : (e + 1) * 128],
                            ident[:],
                        )
                    dst = xT_sc[:, e * 1024 : (e + 1) * 1024]
                    if e % 2 == 0:
                        nc.vector.tensor_copy(dst, p[:])
                    else:
                        nc.scalar.copy(dst, p[:])

                m1 = m1p.tile([128, 1024], F32)
                m2 = m2p.tile([128, 1024], F32)
                for e in range(ETILES):
                    lhs1 = w_sb[:, e * WSTR : e * WSTR + 128]
                    lhs2 = w_sb[:, e * WSTR + 128 : e * WSTR + 256]
                    for n in range(2):
                        rhs = xT_sc[:, e * 1024 + n * 512 : e * 1024 + (n + 1) * 512]
                        nc.tensor.matmul(
                            m1[:, n * 512 : (n + 1) * 512], lhs1, rhs,
                            start=(e == 0), stop=(e == ETILES - 1),
                        )
                        nc.tensor.matmul(
                            m2[:, n * 512 : (n + 1) * 512], lhs2, rhs,
                            start=(e == 0), stop=(e == ETILES - 1),
                        )
                kq = stg.tile([128, 1024], BF16, tag="kq")
                nc.vector.tensor_scalar_add(kq[:], m1[:], bkq[:])
                vst = stg.tile([64, 1024], BF16, tag="vst")
                nc.scalar.add(vst[:], m2[0:64, :], bv[:])

                # stage into the AG and keep qT locally
                nc.gpsimd.dma_start(cc_in[sc][0, 0 : 64 * 1024], kq[0:64, :])
                nc.gpsimd.dma_start(cc_in[sc][0, 64 * 1024 : WKV], vst[:])
                nc.sync.dma_start(
                    q2_sb[0:64, sc * 1024 : (sc + 1) * 1024], kq[64:128, :]
                )
                nc.gpsimd.collective_compute(
                    "AllGather",
                    ALU.bypass,
                    replica_groups=REPLICA_GROUPS,
                    ins=[cc_in[sc].opt()],
                    outs=[cc_out[sc].opt()],
                )
                # unpack both halves in global order; r = global half index
                for r in range(2):
                    cols = slice(r * SH + sc * 1024, r * SH + (sc + 1) * 1024)
                    nc.sync.dma_start(
                        kt_sb[0:64, cols],
                        cc_out[sc][r, 0 : 64 * 1024].rearrange("(p f) -> p f", p=64),
                    )
                    nc.sync.dma_start(
                        vT_sb[:, cols],
                        cc_out[sc][r, 64 * 1024 : WKV].rearrange("(p f) -> p f", p=64),
                    )
                # v natural tiles covered by this AG chunk
                for r in range(2):
                    for t in range(8):
                        kj = r * 16 + sc * 8 + t
                        p = vps.tile([128, 64], BF16)
                        nc.tensor.transpose(
                            p[:],
                            vT_sb[:, kj * 128 : (kj + 1) * 128],
                            ident[0:64, 0:64],
                        )
                        nc.vector.tensor_copy(
                            v_aug[:, kj * 128 : kj * 128 + 64], p[:]
                        )

        # Phase-B-only constants, emitted late so they never stall phase A.
        nc.gpsimd.memset(kt_sb[64:128, :], 0.0)
        nc.gpsimd.memset(ones11[:], 1.0)
        # ones column (index 64) of every kj tile in one strided memset; the
        # junk columns 65:128 only feed unused ctx rows
        nc.gpsimd.memset(
            v_aug[:].rearrange("p (t c) -> p t c", c=128)[:, :, 64:65], 1.0
        )
        # W_out padded: rows 0:64 = W_out, row 64 = b_out, rows 65:128 = 0
        wo32 = wstage.tile([H, E], F32)
        nc.gpsimd.dma_start(wo32[:], wo_ext[:, :])
        bo16 = wstage.tile([1, E], BF16)
        bo32 = wstage.tile([1, E], F32)
        nc.gpsimd.dma_start(bo32[:], bo_ext.unsqueeze(0))
        nc.vector.tensor_copy(bo16[:], bo32[:])
        nc.gpsimd.memset(wo_sb[:], 0.0)
        nc.vector.tensor_copy(wo_sb[0:64, :], wo32[:])
        nc.gpsimd.dma_start(wo_sb[64:65, :], bo16[:])
        wstage_ctx.close()

        # kj visit order: tiles covered by AG#0 first, then AG#1
        kj_order = (
            list(range(0, 8)) + list(range(16, 24))
            + list(range(8, 16)) + list(range(24, 32))
        )

        # ---- Phase B/C: attention + output projection -------------------
        with ExitStack() as pb:
            sps = pb.enter_context(tc.tile_pool(name="sps", bufs=2, space="PSUM"))
            cps = pb.enter_context(tc.tile_pool(name="cps", bufs=1, space="PSUM"))
            ops = pb.enter_context(tc.tile_pool(name="ops", bufs=2, space="PSUM"))
            expp = pb.enter_context(tc.tile_pool(name="expp", bufs=4))
            ctxp = pb.enter_context(tc.tile_pool(name="ctxp", bufs=2))
            rsp = pb.enter_context(tc.tile_pool(name="rsp", bufs=2))
            outp = pb.enter_context(tc.tile_pool(name="outp", bufs=3))

            for qc in range(SH // QC):
                q0 = qc * QC
                ctx = cps.tile([128, QC], F32)  # rows 0:64 ctx, row 64 denom
                for i, kj in enumerate(kj_order):
                    sc_ps = sps.tile([128, QC], F32)
                    lhs_k = kt_sb[:, kj * 128 : (kj + 1) * 128]
                    for n in range(QC // 512):
                        nc.tensor.matmul(
                            sc_ps[:, n * 512 : (n + 1) * 512],
                            lhs_k,
                            q2_sb[:, q0 + n * 512 : q0 + (n + 1) * 512],
                        )
                    ex = expp.tile([128, QC], BF16)
                    nc.scalar.activation(ex[:], sc_ps[:], AF.Exp, scale=SCALE)
                    lhs_v = v_aug[:, kj * 128 : (kj + 1) * 128]
                    for n in range(QC // 512):
                        nc.tensor.matmul(
                            ctx[:, n * 512 : (n + 1) * 512],
                            lhs_v,
                            ex[:, n * 512 : (n + 1) * 512],
                            start=(i == 0), stop=(i == ST - 1),
                            skip_group_check=True,
                        )

                ctx_sb = ctxp.tile([65, QC], F32, tag="ctx32")
                nc.vector.tensor_copy(ctx_sb[:], ctx[0:65, :])
                # bf16 context with the denominator kept as row 64 (so the
                # b_out row of wo_sb gets scaled by denom*recip == 1)
                ctx_b16 = ctxp.tile([128, QC], BF16, tag="ctx16")
                nc.gpsimd.memset(ctx_b16[64:128, :], 0.0)
                nc.vector.tensor_copy(ctx_b16[0:65, :], ctx_sb[:])
                rs_row = rsp.tile([1, QC], F32, tag="rsrow")
                nc.sync.dma_start(rs_row[:], ctx_sb[64:65, :])

                rs_ps = ops.tile([128, QC // 128], F32, tag="op")
                for c in range(QC // 128):
                    nc.tensor.matmul(
                        rs_ps[:, c : c + 1],
                        rs_row[0:1, c * 128 : (c + 1) * 128],
                        ones11[:],
                    )
                recip = rsp.tile([128, QC // 128], F32, tag="recip")
                nc.vector.reciprocal(recip[:], rs_ps[:])

                for c in range(QC // 128):
                    out_sb = outp.tile([128, E], F32)
                    for n in range(2):
                        op = ops.tile([128, 512], F32, tag="op")
                        nc.tensor.matmul(
                            op[:],
                            ctx_b16[:, c * 128 : (c + 1) * 128],
                            wo_sb[:, n * 512 : (n + 1) * 512],
                        )
                        if (c + n) % 2 == 0:
                            nc.vector.tensor_scalar_mul(
                                out_sb[:, n * 512 : (n + 1) * 512],
                                op[:],
                                recip[:, c : c + 1],
                            )
                        else:
                            nc.scalar.mul(
                                out_sb[:, n * 512 : (n + 1) * 512],
                                op[:],
                                recip[:, c : c + 1],
                            )
                    nc.sync.dma_start(
                        out_ext[q0 + c * 128 : q0 + (c + 1) * 128, :], out_sb[:]
                    )


_NC = None


def _get_nc():
    global _NC
    if _NC is None:
        nc = bacc.Bacc("TRN2", target_bir_lowering=False, debug=False,
                       num_devices=N_CORES)
        x_ext = nc.dram_tensor("x", [SH, E], F32, kind="ExternalInput").ap()
        wq_ext = nc.dram_tensor("w_qkv", [E, D3], F32, kind="ExternalInput").ap()
        bq_ext = nc.dram_tensor("b_qkv", [D3], F32, kind="ExternalInput").ap()
        wo_ext = nc.dram_tensor("w_out", [H, E], F32, kind="ExternalInput").ap()
        bo_ext = nc.dram_tensor("b_out", [E], F32, kind="ExternalInput").ap()
        out_ext = nc.dram_tensor("out", [SH, E], F32, kind="ExternalOutput").ap()
        with tile.TileContext(nc) as tc:
            _emit(nc, tc, x_ext, wq_ext, bq_ext, wo_ext, bo_ext, out_ext)
        nc.compile()
        _NC = nc
    return _NC


last_results = None
last_tmpdir = None


def kernel(x, W_qkv, b_qkv, W_out, b_out):
    nc = _get_nc()
    x = np.ascontiguousarray(x, dtype=np.float32)
    shared = {
        "w_qkv": np.ascontiguousarray(W_qkv, dtype=np.float32),
        "b_qkv": np.ascontiguousarray(b_qkv, dtype=np.float32),
        "w_out": np.ascontiguousarray(W_out, dtype=np.float32),
        "b_out": np.ascontiguousarray(b_out, dtype=np.float32),
    }
    in_maps = []
    for c in range(N_CORES):
        b, h = divmod(c, 2)
        xp = np.ascontiguousarray(x[b, h * SH : (h + 1) * SH])
        in_maps.append({"x": xp, **shared})

    import os
    import tempfile

    tmpdir = os.environ.get("ATTN_TRACE_DIR") or tempfile.mkdtemp(prefix="attn_trace_")
    res = run_bass_kernel_spmd(nc, in_maps, core_ids=list(range(N_CORES)), tmpdir=tmpdir)
    global last_results, last_tmpdir
    last_results = res
    last_tmpdir = tmpdir

    out = np.empty((B, S, E), dtype=np.float32)
    for c in range(N_CORES):
        b, h = divmod(c, 2)
        out[b, h * SH : (h + 1) * SH] = res.results[c]["out"]
    return out
